# revision 36
# baseline (speedup 1.0000x reference)
"""TRN2 Bass kernel for nn_CNActorController (GRU comm-net actor).

Strategy (self-contained, shapes hardcoded):
- Pure data parallelism: batch dim sharded 8 ways (1024 samples / core).
- On-chip layout is feature-major ("transposed"): activations live as
  [feature partitions, row free-dim], rows ordered sample-major (b*16+a).
- The agent-mean communication step collapses (mask==const across agents):
    ci_n @ gru_k = (sum_a x)/16 @ gru_k + const
  so it is computed per-sample via a segmented row-sum + a small matmul,
  and all OU-noise terms + biases collapse into per-gate constants folded
  on the host.
- fp16 on-chip datatypes throughout (1 cyc/row on PE, 2x DVE mode),
  fp32 PSUM accumulation.
- Work is spread across all four compute engines: PE (matmuls), ACT
  (sigmoid/tanh/relu evacuations), DVE (gate combine arithmetic + some
  psum evacuations), Pool/GPSIMD (segment sums + broadcast adds).
"""
import os
import numpy as np

import jax
try:
    _cache_dir = os.path.expanduser("~/.cache/jax_kernel_cache")
    os.makedirs(_cache_dir, exist_ok=True)
    jax.config.update("jax_compilation_cache_dir", _cache_dir)
    jax.config.update("jax_persistent_cache_min_entry_size_bytes", -1)
    jax.config.update("jax_persistent_cache_min_compile_time_secs", 2)
except Exception:
    pass

import concourse.bass as bass
import concourse.mybir as mybir
import concourse.tile as tile
from concourse import bacc
from concourse.bass_utils import run_bass_kernel_spmd

# problem dims (hardcoded per spec)
B, A, D, H, C, NA = 8192, 16, 64, 256, 2, 5
NCORES = 8
B_LOC = B // NCORES          # 1024 samples per core
R_LOC = B_LOC * A            # 16384 rows per core
P = 128

SC_SAMPLES = 256             # samples per super-chunk
N_SC = B_LOC // SC_SAMPLES   # 4 super-chunks
CHUNK = 512                  # rows per chunk
CH_S = CHUNK // A            # 32 samples per chunk
N_CH = SC_SAMPLES * A // CHUNK  # 8 chunks per super-chunk
SC_ROWS = SC_SAMPLES * A     # 4096 rows per super-chunk

MT_G = 6                     # gate m-tiles (3H/128)
MT_H = 2                     # state m-tiles (H/128)
KT = 2                       # k-tiles for H contraction

f32 = mybir.dt.float32
f16 = mybir.dt.float16
AF = mybir.ActivationFunctionType
ALU = mybir.AluOpType

_CACHE = {}
N_REPS = 1

# engine placement knobs (tuned via timeline sim)
SEG_ENGINES = "pppp"         # segsum tree levels: p=pool, v=dve per level
HP_ENGINE = "dve"           # h-gate broadcast add: "pool" | "dve"
DEC_ENGINE = "dve"         # dec relu evac: "dve" | "act" | "split"
CMT_ENGINE = "dve"           # cmt psum evac: "dve" | "act"
ENC2_ENGINE = "split"          # enc2 relu evac: "dve" | "act"
CM_L1_ENGINE = "act"         # layer-1 cm evac: "dve" | "act"
XOUT_L1_ENGINE = "dve"      # layer-1 final +h0: "pool" | "dve"
INTERLEAVE = True            # emit barrier blocks inside chunk loops
STREAM_OFF = 12              # slot offset between superchunk streams (24=serial)
SC_BUFS = 2                  # h0/x1 ring
TP_BUFS = 4                  # transient ring
PG_BUFS, PM_BUFS, PT_BUFS = 4, 2, 2   # psum bank split (sum*banks <= 8)


def _segsum_tree(nc, pool, engs, dst, src, tag):
    """dst[p, kt, s] = sum_a src[p, kt, s, a] via log2(A) adds.

    src is an AP [P, KT, A, S] (agent-major rows); engs is 4 engines, one
    per tree level (mix of nc.gpsimd / nc.vector — 2-byte packed operands
    let DVE run the adds in 2x mode, unlike tensor_reduce).
    """
    s = src.shape[3]
    t8 = pool.tile([P, KT, 8, s], f16, name=f"t8_{tag}", tag="segt8")
    t4 = pool.tile([P, KT, 4, s], f16, name=f"t4_{tag}", tag="segt4")
    t2 = pool.tile([P, KT, 2, s], f16, name=f"t2_{tag}", tag="segt2")
    engs[0].tensor_tensor(t8[:], src[:, :, 0:8, :], src[:, :, 8:16, :],
                          op=ALU.add)
    engs[1].tensor_tensor(t4[:], t8[:, :, 0:4, :], t8[:, :, 4:8, :],
                          op=ALU.add)
    engs[2].tensor_tensor(t2[:], t4[:, :, 0:2, :], t4[:, :, 2:4, :],
                          op=ALU.add)
    engs[3].tensor_tensor(dst, t2[:, :, 0, :], t2[:, :, 1, :], op=ALU.add)


def _build():
    """Build the per-core Bass module (same program for all 8 cores)."""
    nc = bacc.Bacc("TRN2", target_bir_lowering=False, debug=False,
                   num_devices=NCORES)

    # ---- DRAM I/O (all weights pre-converted to fp16 on host) ----
    obs_d = nc.dram_tensor("obs", [D, R_LOC], f16, kind="ExternalInput")
    wenc_d = nc.dram_tensor("wenc", [D, H], f16, kind="ExternalInput")
    benc_d = nc.dram_tensor("benc", [H], f32, kind="ExternalInput")
    wenc2_d = nc.dram_tensor("wenc2", [H, H], f16, kind="ExternalInput")
    benc2_d = nc.dram_tensor("benc2", [H], f32, kind="ExternalInput")
    grk_d = nc.dram_tensor("grk", [C, H, 3 * H], f16, kind="ExternalInput")
    gks_d = nc.dram_tensor("gks", [C, H, 3 * H], f16, kind="ExternalInput")
    cmb_d = nc.dram_tensor("cmb", [C, 3 * H], f32, kind="ExternalInput")
    b1h_d = nc.dram_tensor("b1h", [C, H], f32, kind="ExternalInput")
    wdec_d = nc.dram_tensor("wdec", [H, H], f16, kind="ExternalInput")
    bdec_d = nc.dram_tensor("bdec", [H], f32, kind="ExternalInput")
    wout_d = nc.dram_tensor("wout", [H, NA], f16, kind="ExternalInput")
    bout_d = nc.dram_tensor("bout", [NA, 1], f32, kind="ExternalInput")
    e32_d = nc.dram_tensor("e32", [P, CHUNK], f16, kind="ExternalInput")
    out_d = nc.dram_tensor("out_t", [NA, R_LOC], f16, kind="ExternalOutput")

    with tile.TileContext(nc) as tc:
        with (
            tc.tile_pool(name="wp", bufs=1) as wp,          # weights, consts
            tc.tile_pool(name="sc", bufs=SC_BUFS) as scp,   # per-superchunk h0
            tc.tile_pool(name="xp", bufs=SC_BUFS) as xp,    # x state buffer
            tc.tile_pool(name="sp", bufs=2) as sp,          # segsums + cm
            tc.tile_pool(name="tp", bufs=TP_BUFS) as tp,    # chunk transients
            tc.tile_pool(name="pg", bufs=PG_BUFS, space="PSUM") as pg,
            tc.tile_pool(name="pm", bufs=PM_BUFS, space="PSUM") as pm,
            tc.tile_pool(name="pt", bufs=PT_BUFS, space="PSUM") as pt,
        ):
            # "mm" ring: enc/enc2/dec/out evac psums; "bar" ring: layer
            # boundary (cm_h blocks + cmt transposes); gates get their own
            # ring so the PE can run ahead of slow evacuations.
            def psum_mm(name):
                return pm.tile([P, CHUNK], f32, name=name, tag="mm")

            def psum_bar(name):
                return pt.tile([P, CHUNK], f32, name=name, tag="bar")
            # ---- load weights (direct fp16 DMA, no rounding copies) ----
            wenc = wp.tile([D, H], f16, name="wenc")
            nc.sync.dma_start(wenc[:], wenc_d[:])
            wenc2 = wp.tile([P, KT, H], f16, name="wenc2")
            nc.sync.dma_start(wenc2[:],
                              wenc2_d.rearrange("(kt p) m -> p kt m", p=P))
            grk = wp.tile([P, C, KT, 3 * H], f16, name="grk")
            gks = wp.tile([P, C, KT, 3 * H], f16, name="gks")
            for li in range(C):
                nc.sync.dma_start(
                    grk[:, li], grk_d[li].rearrange("(kt p) m -> p kt m", p=P))
                nc.sync.dma_start(
                    gks[:, li], gks_d[li].rearrange("(kt p) m -> p kt m", p=P))
            wdec = wp.tile([P, KT, H], f16, name="wdec")
            nc.sync.dma_start(wdec[:],
                              wdec_d.rearrange("(kt p) m -> p kt m", p=P))
            wout = wp.tile([P, KT, NA], f16, name="wout")
            nc.sync.dma_start(wout[:],
                              wout_d.rearrange("(kt p) m -> p kt m", p=P))

            benc = wp.tile([P, MT_H], f32, name="benc")
            nc.sync.dma_start(benc[:], benc_d.rearrange("(mt p) -> p mt", p=P))
            benc2 = wp.tile([P, MT_H], f32, name="benc2")
            nc.sync.dma_start(benc2[:], benc2_d.rearrange("(mt p) -> p mt", p=P))
            bdec = wp.tile([P, MT_H], f32, name="bdec")
            nc.sync.dma_start(bdec[:], bdec_d.rearrange("(mt p) -> p mt", p=P))
            bout = wp.tile([NA, 1], f32, name="bout")
            nc.sync.dma_start(bout[:], bout_d[:])
            cmb = wp.tile([P, C, MT_G], f32, name="cmb")
            nc.sync.dma_start(cmb[:], cmb_d.rearrange("c (mt p) -> p c mt", p=P))
            b1h = wp.tile([P, C, MT_H], f32, name="b1h")
            nc.sync.dma_start(b1h[:], b1h_d.rearrange("c (mt p) -> p c mt", p=P))
            e32 = wp.tile([P, CHUNK], f16, name="e32")
            nc.sync.dma_start(e32[:], e32_d[:])

            # ---- main loop over super-chunks (software-pipelined
            # emission: engines execute their streams in order, so each
            # layer's cm/cmt "barrier" blocks are emitted right after the
            # two chunks they depend on) ----
            BS = 2 * CH_S            # 64 samples per barrier block
            seg_engs = [nc.gpsimd if c == "p" else nc.vector
                        for c in SEG_ENGINES]

            def emit_phase1_chunk(sc, ctx, ch):
                h0, s1 = ctx["h0"], ctx["s1"]
                r0 = sc * SC_ROWS + ch * CHUNK   # global row offset
                co = ch * CHUNK                  # row offset within sc
                obs_t = tp.tile([D, CHUNK], f16, name="obs_t")
                nc.sync.dma_start(obs_t[:], obs_d[:, r0:r0 + CHUNK])
                # enc (K=64), then enc2 (K=256)
                x1t = tp.tile([P, MT_H, CHUNK], f16, name="x1t", tag="mlp_tmp")
                for mt in range(MT_H):
                    pse = psum_mm("pse")
                    nc.tensor.matmul(
                        pse[:], wenc[:, mt * P:(mt + 1) * P], obs_t[:],
                        start=True, stop=True)
                    nc.scalar.activation(
                        x1t[:, mt, :], pse[:], AF.Relu, bias=benc[:, mt:mt + 1])
                for mt in range(MT_H):
                    pse2 = psum_mm("pse2")
                    for kt in range(KT):
                        nc.tensor.matmul(
                            pse2[:], wenc2[:, kt, mt * P:(mt + 1) * P],
                            x1t[:, kt, :], start=(kt == 0), stop=(kt == KT - 1))
                    e2_dve = (ENC2_ENGINE == "dve" or
                              (ENC2_ENGINE == "split" and mt == 1))
                    if e2_dve:
                        nc.vector.tensor_scalar(
                            h0[:, mt, co:co + CHUNK], pse2[:],
                            benc2[:, mt:mt + 1], 0.0, op0=ALU.add, op1=ALU.max)
                    else:
                        nc.scalar.activation(
                            h0[:, mt, co:co + CHUNK], pse2[:], AF.Relu,
                            bias=benc2[:, mt:mt + 1])
                # segsum of h0 chunk -> s1 (tree levels split Pool/DVE)
                _segsum_tree(
                    nc, tp, seg_engs,
                    s1[:, :, ch * CH_S:(ch + 1) * CH_S],
                    h0[:, :, co:co + CHUNK].rearrange(
                        "p mt (a s) -> p mt a s", s=CH_S),
                    tag="s1")

            def emit_barrier_block(ctx, li, blk):
                """cm_h block for 64 samples; every second call also emits
                the transposed z,r cm for a packed 128-sample group."""
                sseg = ctx["s1"] if li == 0 else ctx["s2"]
                cm = ctx["cm"][li]
                psc = psum_bar("psc")
                for mt in range(MT_H):
                    for kt in range(KT):
                        nc.tensor.matmul(
                            psc[:, mt * BS:(mt + 1) * BS],
                            gks[:, li, kt, (4 + mt) * P:(5 + mt) * P],
                            sseg[:, kt, blk * BS:(blk + 1) * BS],
                            start=(kt == 0), stop=(kt == KT - 1))
                    if li == 0 or CM_L1_ENGINE == "act":
                        nc.scalar.activation(
                            cm[:, mt, blk * BS:(blk + 1) * BS],
                            psc[:, mt * BS:(mt + 1) * BS], AF.Identity,
                            bias=cmb[:, li, 4 + mt:5 + mt])
                    else:
                        nc.vector.tensor_scalar(
                            cm[:, mt, blk * BS:(blk + 1) * BS],
                            psc[:, mt * BS:(mt + 1) * BS],
                            cmb[:, li, 4 + mt:5 + mt], None, op0=ALU.add)
                if blk % 2 == 1:
                    # transposed cm for z,r: 128 samples per stationary load
                    # (evacuated as two 64-partition tiles; PE APs only
                    # support base partitions 0/32/64)
                    g4 = blk // 2
                    pct = psum_bar("pct")
                    for kt in range(KT):
                        nc.tensor.matmul(
                            pct[:],
                            sseg[:, kt, g4 * P:(g4 + 1) * P],
                            gks[:, li, kt, 0:2 * H],
                            start=(kt == 0), stop=(kt == KT - 1))
                    for half in range(2):
                        cmt = sp.tile([64, 2 * H], f16, name="cmt", tag="cmt",
                                      bufs=8)
                        if CMT_ENGINE == "dve":
                            nc.vector.tensor_copy(
                                cmt[:], pct[half * 64:(half + 1) * 64, :])
                        else:
                            nc.scalar.copy(
                                cmt[:], pct[half * 64:(half + 1) * 64, :])
                        ctx["cmts"][li].append(cmt)

            def emit_gru_chunk(sc, ctx, li, ch):
                h0, s1, s2, x1 = ctx["h0"], ctx["s1"], ctx["s2"], ctx["x1"]
                cm, cmts = ctx["cm"][li], ctx["cmts"][li]
                xin = h0 if li == 0 else x1
                xout = x1
                co = ch * CHUNK
                ss = ch * CH_S
                zt = tp.tile([P, MT_H, CHUNK], f16, name="zt")
                rt = tp.tile([P, MT_H, CHUNK], f16, name="rt")
                hh = tp.tile([P, MT_H, CHUNK], f16, name="hh")
                # gate psums: mh = x @ gru_rk.  m-tile order r,h,z: the
                # critical chain runs r-sigmoid -> rrh -> hp -> tanh ->
                # combine, so r first and z (only needed at the combine
                # multiply) last.
                rrh = tp.tile([P, MT_H, CHUNK], f16, name="rrh")
                for mt in (2, 3, 4, 5, 0, 1):
                    psg = pg.tile([P, CHUNK], f32, name="psg")
                    for kt in range(KT):
                        nc.tensor.matmul(
                            psg[:], grk[:, li, kt, mt * P:(mt + 1) * P],
                            xin[:, kt, co:co + CHUNK],
                            start=(kt == 0), stop=(kt == KT - 1 and mt >= 4))
                    if mt < 4:
                        # z,r: cm broadcast via expander matmul into psum,
                        # then sigmoid w/ folded const bias
                        po = 32 * (ch % 2)
                        nc.tensor.matmul(
                            psg[:],
                            cmts[ch // 2][po:po + 32, mt * P:(mt + 1) * P],
                            e32[po:po + 32, :], start=False, stop=True)
                        dst = zt if mt < 2 else rt
                        nc.scalar.activation(
                            dst[:, mt % 2, :], psg[:], AF.Sigmoid,
                            bias=cmb[:, li, mt:mt + 1])
                    else:
                        # h gate: rrh = r * (mh_h + b1h) per m-tile
                        mtl = mt - 4
                        nc.vector.scalar_tensor_tensor(
                            rrh[:, mtl, :], psg[:], b1h[:, li, mtl:mtl + 1],
                            rt[:, mtl, :], op0=ALU.add, op1=ALU.mult)
                # merged across both h m-tiles: one broadcast add on Pool
                # + one tanh on ACT
                cmv2 = cm[:, :, ss:ss + CH_S].unsqueeze(
                    2).broadcast_to([P, MT_H, A, CH_S])
                hp = tp.tile([P, MT_H, CHUNK], f16, name="hp")
                eng_hp = nc.gpsimd if HP_ENGINE == "pool" else nc.vector
                eng_hp.tensor_tensor(
                    hp.rearrange("p mt (a s) -> p mt a s", s=CH_S),
                    rrh.rearrange("p mt (a s) -> p mt a s", s=CH_S),
                    cmv2, op=ALU.add)
                nc.scalar.activation(hh[:], hp[:], AF.Tanh)
                # combine: x_new = z*(x - hh) + (hh + h0); the hh+h0 term
                # runs on Pool in parallel with the DVE sub/mult chain
                dtl = tp.tile([P, MT_H, CHUNK], f16, name="dtl")
                hpl = tp.tile([P, MT_H, CHUNK], f16, name="hpl")
                nc.vector.tensor_tensor(
                    dtl[:], xin[:, :, co:co + CHUNK], hh[:], op=ALU.subtract)
                nc.gpsimd.tensor_tensor(
                    hpl[:], hh[:], h0[:, :, co:co + CHUNK], op=ALU.add)
                nc.vector.tensor_tensor(dtl[:], zt[:], dtl[:], op=ALU.mult)
                nc.vector.tensor_tensor(
                    xout[:, :, co:co + CHUNK], dtl[:], hpl[:], op=ALU.add)
                if li == 0:
                    _segsum_tree(
                        nc, tp, seg_engs,
                        s2[:, :, ss:ss + CH_S],
                        xout[:, :, co:co + CHUNK].rearrange(
                            "p mt (a s) -> p mt a s", s=CH_S),
                        tag="s2")
                else:
                    # ---- dec + out fused ----
                    dd = tp.tile([P, MT_H, CHUNK], f16, name="dd")
                    for mt in range(MT_H):
                        psd = psum_mm("psd")
                        for kt in range(KT):
                            nc.tensor.matmul(
                                psd[:], wdec[:, kt, mt * P:(mt + 1) * P],
                                xout[:, kt, co:co + CHUNK],
                                start=(kt == 0), stop=(kt == KT - 1))
                        dec_act = (DEC_ENGINE == "act" or
                                   (DEC_ENGINE == "split" and mt == 0))
                        if dec_act:
                            nc.scalar.activation(
                                dd[:, mt, :], psd[:], AF.Relu,
                                bias=bdec[:, mt:mt + 1])
                        else:
                            nc.vector.tensor_scalar(
                                dd[:, mt, :], psd[:], bdec[:, mt:mt + 1], 0.0,
                                op0=ALU.add, op1=ALU.max)
                    pso = pm.tile([NA, CHUNK], f32, name="pso", tag="mm")
                    for kt in range(KT):
                        nc.tensor.matmul(
                            pso[:], wout[:, kt, :], dd[:, kt, :],
                            start=(kt == 0), stop=(kt == KT - 1))
                    ot = tp.tile([NA, CHUNK], f16, name="ot")
                    nc.scalar.add(ot[:], pso[:], bout[:])
                    nc.sync.dma_start(
                        out_d[:, sc * SC_ROWS + co:sc * SC_ROWS + co + CHUNK],
                        ot[:])

            import contextlib
            rep_ctx = (tc.For_i(0, N_REPS, 1) if N_REPS > 1
                       else contextlib.nullcontext())
            def make_ctx():
                return {
                    "h0": scp.tile([P, MT_H, SC_ROWS], f16, name="h0",
                                   tag="h0"),
                    "s1": sp.tile([P, KT, SC_SAMPLES], f16, name="s1",
                                  tag="seg1"),
                    "s2": sp.tile([P, KT, SC_SAMPLES], f16, name="s2",
                                  tag="seg2"),
                    "x1": xp.tile([P, MT_H, SC_ROWS], f16, name="x1",
                                  tag="x"),
                    "cm": [sp.tile([P, MT_H, SC_SAMPLES], f16,
                                   name=f"cm{li}", tag="cm", bufs=4)
                           for li in range(C)],
                    "cmts": [[], []],
                }

            def emit_step(sc, ctx, s):
                # 24 steps per superchunk: 8 phase-1, 8 layer-0, 8 layer-1;
                # barrier blocks ride the odd steps of the preceding phase.
                ph, ch = divmod(s, N_CH)
                if ph == 0:
                    emit_phase1_chunk(sc, ctx, ch)
                    if ch % 2 == 1:
                        emit_barrier_block(ctx, 0, (ch - 1) // 2)
                elif ph == 1:
                    emit_gru_chunk(sc, ctx, 0, ch)
                    if ch % 2 == 1:
                        emit_barrier_block(ctx, 1, (ch - 1) // 2)
                else:
                    emit_gru_chunk(sc, ctx, 1, ch)

            N_STEP = 3 * N_CH
            with rep_ctx:
              ctxs = {}
              for t in range((N_SC - 1) * STREAM_OFF + N_STEP):
                  for sc in range(N_SC):
                      s = t - sc * STREAM_OFF
                      if s == 0:
                          ctxs[sc] = make_ctx()
                      if 0 <= s < N_STEP:
                          emit_step(sc, ctxs[sc], s)

    nc.compile()
    return nc


def _host_prep(inputs):
    """Host-side preprocessing of weights/constants (tiny, O(H^2))."""
    g = lambda k: np.asarray(inputs[k], np.float32)
    obs = g("obs")
    mask = g("mask")            # (1, A, 1)
    ou_s0, ou_s1 = g("ou_s0"), g("ou_s1")   # (C,1,A,H)
    ou_s2, ou_s3 = g("ou_s2"), g("ou_s3")   # (C,1,1,H)
    gru_k, gru_b = g("gru_k"), g("gru_b")

    m = mask[0, :, :]                        # (A, 1)
    cmb = np.zeros((C, 3 * H), np.float32)
    b1h = np.zeros((C, H), np.float32)
    gks = np.zeros((C, H, 3 * H), np.float32)
    for i in range(C):
        send = (m * ou_s1[i, 0] * ou_s0[i, 0]).sum(0) / A      # (H,)
        recv = (m.mean(0) * ou_s3[i, 0, 0] * ou_s2[i, 0, 0])   # (H,)
        const = (send + recv).astype(np.float64) @ gru_k[i].astype(np.float64)
        cmbi = const + gru_b[i, 0].astype(np.float64)
        cmbi[:2 * H] += gru_b[i, 1, :2 * H].astype(np.float64)
        cmb[i] = cmbi.astype(np.float32)
        b1h[i] = gru_b[i, 1, 2 * H:]
        gks[i] = gru_k[i] / A

    h16 = lambda x: np.ascontiguousarray(x.astype(np.float16))
    shared = {
        "wenc": h16(g("W_enc")), "benc": g("b_enc"),
        "wenc2": h16(g("W_enc2")), "benc2": g("b_enc2"),
        "grk": h16(g("gru_rk")), "gks": h16(gks), "cmb": cmb, "b1h": b1h,
        "wdec": h16(g("W_dec")), "bdec": g("b_dec"),
        "wout": h16(g("W_out")), "bout": g("b_out").reshape(NA, 1),
    }
    # expander matrix: broadcast per-sample rows to per-row positions
    # (rows within a chunk are agent-major: r = a*CH_S + s; 4 identical
    # selector blocks, one per chunk position within a 128-sample group)
    e32 = np.zeros((P, CHUNK), np.float16)
    for g in range(4):
        for s in range(CH_S):
            e32[g * CH_S + s, s::CH_S] = 1.0
    shared["e32"] = e32
    # per-core transposed obs, fp16, rows permuted agent-major per chunk
    obs_r = obs.reshape(NCORES, B_LOC // CH_S, CH_S, A, D)
    obs_r = obs_r.transpose(0, 1, 3, 2, 4)          # chunk, a, s
    obs_r = obs_r.reshape(NCORES, R_LOC, D).transpose(0, 2, 1)
    obs_r = np.ascontiguousarray(obs_r.astype(np.float16))
    in_maps = [dict(shared, obs=obs_r[k]) for k in range(NCORES)]
    return in_maps


def kernel(**inputs):
    if "nc" not in _CACHE:
        _CACHE["nc"] = _build()
    nc = _CACHE["nc"]
    in_maps = _host_prep(inputs)
    res = run_bass_kernel_spmd(nc, in_maps, core_ids=list(range(NCORES)))
    outs = np.stack([res.results[k]["out_t"] for k in range(NCORES)])
    # (8, 5, 16384) -> (8192, 16, 5), undoing the per-chunk agent-major
    # row permutation (r = a*CH_S + s within each 512-row chunk)
    out = outs.astype(np.float32).reshape(
        NCORES, NA, B_LOC // CH_S, A, CH_S)
    out = out.transpose(0, 2, 4, 3, 1)              # core, chunk, s, a, na
    return np.ascontiguousarray(out.reshape(B, A, NA)).astype(np.float32)


# revision 37
# speedup vs baseline: 1.0562x; 1.0562x over previous
"""TRN2 Bass kernel for nn_CNActorController (GRU comm-net actor).

Strategy (self-contained, shapes hardcoded):
- Pure data parallelism: batch dim sharded 8 ways (1024 samples / core).
- On-chip layout is feature-major ("transposed"): activations live as
  [feature partitions, row free-dim], rows ordered sample-major (b*16+a).
- The agent-mean communication step collapses (mask==const across agents):
    ci_n @ gru_k = (sum_a x)/16 @ gru_k + const
  so it is computed per-sample via a segmented row-sum + a small matmul,
  and all OU-noise terms + biases collapse into per-gate constants folded
  on the host.
- fp16 on-chip datatypes throughout (1 cyc/row on PE, 2x DVE mode),
  fp32 PSUM accumulation.
- Work is spread across all four compute engines: PE (matmuls), ACT
  (sigmoid/tanh/relu evacuations), DVE (gate combine arithmetic + some
  psum evacuations), Pool/GPSIMD (segment sums + broadcast adds).
"""
import os
import numpy as np

import jax
try:
    _cache_dir = os.path.expanduser("~/.cache/jax_kernel_cache")
    os.makedirs(_cache_dir, exist_ok=True)
    jax.config.update("jax_compilation_cache_dir", _cache_dir)
    jax.config.update("jax_persistent_cache_min_entry_size_bytes", -1)
    jax.config.update("jax_persistent_cache_min_compile_time_secs", 2)
except Exception:
    pass

import concourse.bass as bass
import concourse.mybir as mybir
import concourse.tile as tile
from concourse import bacc
from concourse.bass_utils import run_bass_kernel_spmd

# problem dims (hardcoded per spec)
B, A, D, H, C, NA = 8192, 16, 64, 256, 2, 5
NCORES = 8
B_LOC = B // NCORES          # 1024 samples per core
R_LOC = B_LOC * A            # 16384 rows per core
P = 128

SC_SAMPLES = 256             # samples per super-chunk
N_SC = B_LOC // SC_SAMPLES   # 4 super-chunks
CHUNK = 512                  # rows per chunk
CH_S = CHUNK // A            # 32 samples per chunk
N_CH = SC_SAMPLES * A // CHUNK  # 8 chunks per super-chunk
SC_ROWS = SC_SAMPLES * A     # 4096 rows per super-chunk

MT_G = 6                     # gate m-tiles (3H/128)
MT_H = 2                     # state m-tiles (H/128)
KT = 2                       # k-tiles for H contraction

f32 = mybir.dt.float32
f16 = mybir.dt.float16
AF = mybir.ActivationFunctionType
ALU = mybir.AluOpType

_CACHE = {}
N_REPS = 1

# engine placement knobs (tuned via timeline sim)
SEG_ENGINES = "pppp"         # segsum tree levels: p=pool, v=dve per level
HP_ENGINE = "dve"           # h-gate broadcast add: "pool" | "dve"
DEC_ENGINE = "dve"         # dec relu evac: "dve" | "act" | "split"
CMT_ENGINE = "dve"           # cmt psum evac: "dve" | "act"
ENC2_ENGINE = "split"          # enc2 relu evac: "dve" | "act"
CM_L1_ENGINE = "act"         # layer-1 cm evac: "dve" | "act"
XOUT_L1_ENGINE = "dve"      # layer-1 final +h0: "pool" | "dve"
INTERLEAVE = True            # emit barrier blocks inside chunk loops
STREAM_OFF = 12              # slot offset between superchunk streams (24=serial)
SC_BUFS = 2                  # h0/x1 ring
TP_BUFS = 4                  # transient ring
PG_BUFS, PM_BUFS, PT_BUFS = 4, 2, 2   # psum bank split (sum*banks <= 8)


def _segsum_tree(nc, pool, engs, dst, src, tag):
    """dst[p, kt, s] = sum_a src[p, kt, s, a] via log2(A) adds.

    src is an AP [P, KT, A, S] (agent-major rows); engs is 4 engines, one
    per tree level (mix of nc.gpsimd / nc.vector — 2-byte packed operands
    let DVE run the adds in 2x mode, unlike tensor_reduce).
    """
    s = src.shape[3]
    t8 = pool.tile([P, KT, 8, s], f16, name=f"t8_{tag}", tag="segt8")
    t4 = pool.tile([P, KT, 4, s], f16, name=f"t4_{tag}", tag="segt4")
    t2 = pool.tile([P, KT, 2, s], f16, name=f"t2_{tag}", tag="segt2")
    engs[0].tensor_tensor(t8[:], src[:, :, 0:8, :], src[:, :, 8:16, :],
                          op=ALU.add)
    engs[1].tensor_tensor(t4[:], t8[:, :, 0:4, :], t8[:, :, 4:8, :],
                          op=ALU.add)
    engs[2].tensor_tensor(t2[:], t4[:, :, 0:2, :], t4[:, :, 2:4, :],
                          op=ALU.add)
    engs[3].tensor_tensor(dst, t2[:, :, 0, :], t2[:, :, 1, :], op=ALU.add)


def _build():
    """Build the per-core Bass module (same program for all 8 cores)."""
    nc = bacc.Bacc("TRN2", target_bir_lowering=False, debug=False,
                   num_devices=NCORES)

    # ---- DRAM I/O (all weights pre-converted to fp16 on host) ----
    obs_d = nc.dram_tensor("obs", [D, R_LOC], f16, kind="ExternalInput")
    wenc_d = nc.dram_tensor("wenc", [D, H], f16, kind="ExternalInput")
    benc_d = nc.dram_tensor("benc", [H], f32, kind="ExternalInput")
    wenc2_d = nc.dram_tensor("wenc2", [H, H], f16, kind="ExternalInput")
    benc2_d = nc.dram_tensor("benc2", [H], f32, kind="ExternalInput")
    grk_d = nc.dram_tensor("grk", [C, H, 3 * H], f16, kind="ExternalInput")
    gks_d = nc.dram_tensor("gks", [C, H, 3 * H], f16, kind="ExternalInput")
    cmb_d = nc.dram_tensor("cmb", [C, 3 * H], f32, kind="ExternalInput")
    b1h_d = nc.dram_tensor("b1h", [C, H], f32, kind="ExternalInput")
    wdec_d = nc.dram_tensor("wdec", [H, H], f16, kind="ExternalInput")
    bdec_d = nc.dram_tensor("bdec", [H], f32, kind="ExternalInput")
    wout_d = nc.dram_tensor("wout", [H, NA], f16, kind="ExternalInput")
    bout_d = nc.dram_tensor("bout", [NA, 1], f32, kind="ExternalInput")
    e32_d = nc.dram_tensor("e32", [P, CHUNK], f16, kind="ExternalInput")
    out_d = nc.dram_tensor("out_t", [NA, R_LOC], f16, kind="ExternalOutput")

    with tile.TileContext(nc) as tc:
        with (
            tc.tile_pool(name="wp", bufs=1) as wp,          # weights, consts
            tc.tile_pool(name="sc", bufs=SC_BUFS) as scp,   # per-superchunk h0
            tc.tile_pool(name="xp", bufs=SC_BUFS) as xp,    # x state buffer
            tc.tile_pool(name="sp", bufs=2) as sp,          # segsums + cm
            tc.tile_pool(name="tp", bufs=TP_BUFS) as tp,    # chunk transients
            tc.tile_pool(name="pg", bufs=PG_BUFS, space="PSUM") as pg,
            tc.tile_pool(name="pm", bufs=PM_BUFS, space="PSUM") as pm,
            tc.tile_pool(name="pt", bufs=PT_BUFS, space="PSUM") as pt,
        ):
            # "mm" ring: enc/enc2/dec/out evac psums; "bar" ring: layer
            # boundary (cm_h blocks + cmt transposes); gates get their own
            # ring so the PE can run ahead of slow evacuations.
            def psum_mm(name):
                return pm.tile([P, CHUNK], f32, name=name, tag="mm")

            def psum_bar(name):
                return pt.tile([P, CHUNK], f32, name=name, tag="bar")
            # ---- load weights (direct fp16 DMA, no rounding copies) ----
            wenc = wp.tile([D, H], f16, name="wenc")
            nc.sync.dma_start(wenc[:], wenc_d[:])
            wenc2 = wp.tile([P, KT, H], f16, name="wenc2")
            nc.sync.dma_start(wenc2[:],
                              wenc2_d.rearrange("(kt p) m -> p kt m", p=P))
            grk = wp.tile([P, C, KT, 3 * H], f16, name="grk")
            gks = wp.tile([P, C, KT, 3 * H], f16, name="gks")
            for li in range(C):
                nc.sync.dma_start(
                    grk[:, li], grk_d[li].rearrange("(kt p) m -> p kt m", p=P))
                nc.sync.dma_start(
                    gks[:, li], gks_d[li].rearrange("(kt p) m -> p kt m", p=P))
            wdec = wp.tile([P, KT, H], f16, name="wdec")
            nc.sync.dma_start(wdec[:],
                              wdec_d.rearrange("(kt p) m -> p kt m", p=P))
            wout = wp.tile([P, KT, NA], f16, name="wout")
            nc.sync.dma_start(wout[:],
                              wout_d.rearrange("(kt p) m -> p kt m", p=P))

            benc = wp.tile([P, MT_H], f32, name="benc")
            nc.sync.dma_start(benc[:], benc_d.rearrange("(mt p) -> p mt", p=P))
            benc2 = wp.tile([P, MT_H], f32, name="benc2")
            nc.sync.dma_start(benc2[:], benc2_d.rearrange("(mt p) -> p mt", p=P))
            bdec = wp.tile([P, MT_H], f32, name="bdec")
            nc.sync.dma_start(bdec[:], bdec_d.rearrange("(mt p) -> p mt", p=P))
            bout = wp.tile([NA, 1], f32, name="bout")
            nc.sync.dma_start(bout[:], bout_d[:])
            cmb = wp.tile([P, C, MT_G], f32, name="cmb")
            nc.sync.dma_start(cmb[:], cmb_d.rearrange("c (mt p) -> p c mt", p=P))
            b1h = wp.tile([P, C, MT_H], f32, name="b1h")
            nc.sync.dma_start(b1h[:], b1h_d.rearrange("c (mt p) -> p c mt", p=P))
            e32 = wp.tile([P, CHUNK], f16, name="e32")
            nc.sync.dma_start(e32[:], e32_d[:])

            # ---- main loop over super-chunks (software-pipelined
            # emission: engines execute their streams in order, so each
            # layer's cm/cmt "barrier" blocks are emitted right after the
            # two chunks they depend on) ----
            BS = 2 * CH_S            # 64 samples per barrier block
            seg_engs = [nc.gpsimd if c == "p" else nc.vector
                        for c in SEG_ENGINES]

            def emit_phase1_chunk(sc, ctx, ch):
                h0, s1 = ctx["h0"], ctx["s1"]
                r0 = sc * SC_ROWS + ch * CHUNK   # global row offset
                co = ch * CHUNK                  # row offset within sc
                obs_t = tp.tile([D, CHUNK], f16, name="obs_t")
                nc.sync.dma_start(obs_t[:], obs_d[:, r0:r0 + CHUNK])
                # enc (K=64), then enc2 (K=256)
                x1t = tp.tile([P, MT_H, CHUNK], f16, name="x1t", tag="mlp_tmp")
                for mt in range(MT_H):
                    pse = psum_mm("pse")
                    nc.tensor.matmul(
                        pse[:], wenc[:, mt * P:(mt + 1) * P], obs_t[:],
                        start=True, stop=True)
                    nc.scalar.activation(
                        x1t[:, mt, :], pse[:], AF.Relu, bias=benc[:, mt:mt + 1])
                for mt in range(MT_H):
                    pse2 = psum_mm("pse2")
                    for kt in range(KT):
                        nc.tensor.matmul(
                            pse2[:], wenc2[:, kt, mt * P:(mt + 1) * P],
                            x1t[:, kt, :], start=(kt == 0), stop=(kt == KT - 1))
                    e2_dve = (ENC2_ENGINE == "dve" or
                              (ENC2_ENGINE == "split" and mt == 1))
                    if e2_dve:
                        nc.vector.tensor_scalar(
                            h0[:, mt, co:co + CHUNK], pse2[:],
                            benc2[:, mt:mt + 1], 0.0, op0=ALU.add, op1=ALU.max)
                    else:
                        nc.scalar.activation(
                            h0[:, mt, co:co + CHUNK], pse2[:], AF.Relu,
                            bias=benc2[:, mt:mt + 1])
                # segsum of h0 chunk -> s1 (tree levels split Pool/DVE)
                _segsum_tree(
                    nc, tp, seg_engs,
                    s1[:, :, ch * CH_S:(ch + 1) * CH_S],
                    h0[:, :, co:co + CHUNK].rearrange(
                        "p mt (a s) -> p mt a s", s=CH_S),
                    tag="s1")

            def emit_barrier_block(ctx, li, blk):
                """cm_h block for 64 samples; every second call also emits
                the transposed z,r cm for a packed 128-sample group."""
                sseg = ctx["s1"] if li == 0 else ctx["s2"]
                cm = ctx["cm"][li]
                psc = psum_bar("psc")
                for mt in range(MT_H):
                    for kt in range(KT):
                        nc.tensor.matmul(
                            psc[:, mt * BS:(mt + 1) * BS],
                            gks[:, li, kt, (4 + mt) * P:(5 + mt) * P],
                            sseg[:, kt, blk * BS:(blk + 1) * BS],
                            start=(kt == 0), stop=(kt == KT - 1))
                    if li == 0 or CM_L1_ENGINE == "act":
                        nc.scalar.activation(
                            cm[:, mt, blk * BS:(blk + 1) * BS],
                            psc[:, mt * BS:(mt + 1) * BS], AF.Identity,
                            bias=cmb[:, li, 4 + mt:5 + mt])
                    else:
                        nc.vector.tensor_scalar(
                            cm[:, mt, blk * BS:(blk + 1) * BS],
                            psc[:, mt * BS:(mt + 1) * BS],
                            cmb[:, li, 4 + mt:5 + mt], None, op0=ALU.add)
                if blk % 2 == 1:
                    # transposed cm for z,r: 128 samples per stationary load
                    # (evacuated as two 64-partition tiles; PE APs only
                    # support base partitions 0/32/64)
                    g4 = blk // 2
                    pct = psum_bar("pct")
                    for kt in range(KT):
                        nc.tensor.matmul(
                            pct[:],
                            sseg[:, kt, g4 * P:(g4 + 1) * P],
                            gks[:, li, kt, 0:2 * H],
                            start=(kt == 0), stop=(kt == KT - 1))
                    for half in range(2):
                        cmt = sp.tile([64, 2 * H], f16, name="cmt", tag="cmt",
                                      bufs=8)
                        if CMT_ENGINE == "dve":
                            nc.vector.tensor_copy(
                                cmt[:], pct[half * 64:(half + 1) * 64, :])
                        else:
                            nc.scalar.copy(
                                cmt[:], pct[half * 64:(half + 1) * 64, :])
                        ctx["cmts"][li].append(cmt)

            def emit_gru_chunk(sc, ctx, li, ch):
                h0, s1, s2, x1 = ctx["h0"], ctx["s1"], ctx["s2"], ctx["x1"]
                cm, cmts = ctx["cm"][li], ctx["cmts"][li]
                xin = h0 if li == 0 else x1
                xout = x1
                co = ch * CHUNK
                ss = ch * CH_S
                zt = tp.tile([P, MT_H, CHUNK], f16, name="zt")
                rt = tp.tile([P, MT_H, CHUNK], f16, name="rt")
                hh = tp.tile([P, MT_H, CHUNK], f16, name="hh")
                # gate psums: mh = x @ gru_rk.  m-tile order r,h,z: the
                # critical chain runs r-sigmoid -> rrh -> hp -> tanh ->
                # combine, so r first and z (only needed at the combine
                # multiply) last.
                rrh = tp.tile([P, MT_H, CHUNK], f16, name="rrh")
                for mt in (2, 3, 4, 5, 0, 1):
                    psg = pg.tile([P, CHUNK], f32, name="psg")
                    for kt in range(KT):
                        nc.tensor.matmul(
                            psg[:], grk[:, li, kt, mt * P:(mt + 1) * P],
                            xin[:, kt, co:co + CHUNK],
                            start=(kt == 0), stop=(kt == KT - 1 and mt >= 4))
                    if mt < 4:
                        # z,r: cm broadcast via expander matmul into psum,
                        # then sigmoid w/ folded const bias
                        po = 32 * (ch % 2)
                        nc.tensor.matmul(
                            psg[:],
                            cmts[ch // 2][po:po + 32, mt * P:(mt + 1) * P],
                            e32[po:po + 32, :], start=False, stop=True)
                        dst = zt if mt < 2 else rt
                        nc.scalar.activation(
                            dst[:, mt % 2, :], psg[:], AF.Sigmoid,
                            bias=cmb[:, li, mt:mt + 1])
                    else:
                        # h gate: rrh = r * (mh_h + b1h) per m-tile
                        mtl = mt - 4
                        nc.vector.scalar_tensor_tensor(
                            rrh[:, mtl, :], psg[:], b1h[:, li, mtl:mtl + 1],
                            rt[:, mtl, :], op0=ALU.add, op1=ALU.mult)
                # merged across both h m-tiles: one broadcast add on Pool
                # + one tanh on ACT
                cmv2 = cm[:, :, ss:ss + CH_S].unsqueeze(
                    2).broadcast_to([P, MT_H, A, CH_S])
                hp = tp.tile([P, MT_H, CHUNK], f16, name="hp")
                eng_hp = nc.gpsimd if HP_ENGINE == "pool" else nc.vector
                eng_hp.tensor_tensor(
                    hp.rearrange("p mt (a s) -> p mt a s", s=CH_S),
                    rrh.rearrange("p mt (a s) -> p mt a s", s=CH_S),
                    cmv2, op=ALU.add)
                nc.scalar.activation(hh[:], hp[:], AF.Tanh)
                # combine: x_new = hh + z*(x - hh) + h0
                dtl = tp.tile([P, MT_H, CHUNK], f16, name="dtl")
                nc.vector.tensor_tensor(
                    dtl[:], xin[:, :, co:co + CHUNK], hh[:], op=ALU.subtract)
                nc.vector.tensor_tensor(dtl[:], zt[:], dtl[:], op=ALU.mult)
                nc.vector.tensor_tensor(dtl[:], dtl[:], hh[:], op=ALU.add)
                nc.vector.tensor_tensor(
                    xout[:, :, co:co + CHUNK], dtl[:],
                    h0[:, :, co:co + CHUNK], op=ALU.add)
                if li == 0:
                    _segsum_tree(
                        nc, tp, seg_engs,
                        s2[:, :, ss:ss + CH_S],
                        xout[:, :, co:co + CHUNK].rearrange(
                            "p mt (a s) -> p mt a s", s=CH_S),
                        tag="s2")
                else:
                    # ---- dec + out fused ----
                    dd = tp.tile([P, MT_H, CHUNK], f16, name="dd")
                    for mt in range(MT_H):
                        psd = psum_mm("psd")
                        for kt in range(KT):
                            nc.tensor.matmul(
                                psd[:], wdec[:, kt, mt * P:(mt + 1) * P],
                                xout[:, kt, co:co + CHUNK],
                                start=(kt == 0), stop=(kt == KT - 1))
                        dec_act = (DEC_ENGINE == "act" or
                                   (DEC_ENGINE == "split" and mt == 0))
                        if dec_act:
                            nc.scalar.activation(
                                dd[:, mt, :], psd[:], AF.Relu,
                                bias=bdec[:, mt:mt + 1])
                        else:
                            nc.vector.tensor_scalar(
                                dd[:, mt, :], psd[:], bdec[:, mt:mt + 1], 0.0,
                                op0=ALU.add, op1=ALU.max)
                    pso = pm.tile([NA, CHUNK], f32, name="pso", tag="mm")
                    for kt in range(KT):
                        nc.tensor.matmul(
                            pso[:], wout[:, kt, :], dd[:, kt, :],
                            start=(kt == 0), stop=(kt == KT - 1))
                    ot = tp.tile([NA, CHUNK], f16, name="ot")
                    nc.scalar.add(ot[:], pso[:], bout[:])
                    nc.sync.dma_start(
                        out_d[:, sc * SC_ROWS + co:sc * SC_ROWS + co + CHUNK],
                        ot[:])

            import contextlib
            rep_ctx = (tc.For_i(0, N_REPS, 1) if N_REPS > 1
                       else contextlib.nullcontext())
            def make_ctx():
                return {
                    "h0": scp.tile([P, MT_H, SC_ROWS], f16, name="h0",
                                   tag="h0"),
                    "s1": sp.tile([P, KT, SC_SAMPLES], f16, name="s1",
                                  tag="seg1"),
                    "s2": sp.tile([P, KT, SC_SAMPLES], f16, name="s2",
                                  tag="seg2"),
                    "x1": xp.tile([P, MT_H, SC_ROWS], f16, name="x1",
                                  tag="x"),
                    "cm": [sp.tile([P, MT_H, SC_SAMPLES], f16,
                                   name=f"cm{li}", tag="cm", bufs=4)
                           for li in range(C)],
                    "cmts": [[], []],
                }

            def emit_step(sc, ctx, s):
                # 24 steps per superchunk: 8 phase-1, 8 layer-0, 8 layer-1;
                # barrier blocks ride the odd steps of the preceding phase.
                ph, ch = divmod(s, N_CH)
                if ph == 0:
                    emit_phase1_chunk(sc, ctx, ch)
                    if ch % 2 == 1:
                        emit_barrier_block(ctx, 0, (ch - 1) // 2)
                elif ph == 1:
                    emit_gru_chunk(sc, ctx, 0, ch)
                    if ch % 2 == 1:
                        emit_barrier_block(ctx, 1, (ch - 1) // 2)
                else:
                    emit_gru_chunk(sc, ctx, 1, ch)

            N_STEP = 3 * N_CH
            with rep_ctx:
              ctxs = {}
              for t in range((N_SC - 1) * STREAM_OFF + N_STEP):
                  for sc in range(N_SC):
                      s = t - sc * STREAM_OFF
                      if s == 0:
                          ctxs[sc] = make_ctx()
                      if 0 <= s < N_STEP:
                          emit_step(sc, ctxs[sc], s)

    nc.compile()
    return nc


def _host_prep(inputs):
    """Host-side preprocessing of weights/constants (tiny, O(H^2))."""
    g = lambda k: np.asarray(inputs[k], np.float32)
    obs = g("obs")
    mask = g("mask")            # (1, A, 1)
    ou_s0, ou_s1 = g("ou_s0"), g("ou_s1")   # (C,1,A,H)
    ou_s2, ou_s3 = g("ou_s2"), g("ou_s3")   # (C,1,1,H)
    gru_k, gru_b = g("gru_k"), g("gru_b")

    m = mask[0, :, :]                        # (A, 1)
    cmb = np.zeros((C, 3 * H), np.float32)
    b1h = np.zeros((C, H), np.float32)
    gks = np.zeros((C, H, 3 * H), np.float32)
    for i in range(C):
        send = (m * ou_s1[i, 0] * ou_s0[i, 0]).sum(0) / A      # (H,)
        recv = (m.mean(0) * ou_s3[i, 0, 0] * ou_s2[i, 0, 0])   # (H,)
        const = (send + recv).astype(np.float64) @ gru_k[i].astype(np.float64)
        cmbi = const + gru_b[i, 0].astype(np.float64)
        cmbi[:2 * H] += gru_b[i, 1, :2 * H].astype(np.float64)
        cmb[i] = cmbi.astype(np.float32)
        b1h[i] = gru_b[i, 1, 2 * H:]
        gks[i] = gru_k[i] / A

    h16 = lambda x: np.ascontiguousarray(x.astype(np.float16))
    shared = {
        "wenc": h16(g("W_enc")), "benc": g("b_enc"),
        "wenc2": h16(g("W_enc2")), "benc2": g("b_enc2"),
        "grk": h16(g("gru_rk")), "gks": h16(gks), "cmb": cmb, "b1h": b1h,
        "wdec": h16(g("W_dec")), "bdec": g("b_dec"),
        "wout": h16(g("W_out")), "bout": g("b_out").reshape(NA, 1),
    }
    # expander matrix: broadcast per-sample rows to per-row positions
    # (rows within a chunk are agent-major: r = a*CH_S + s; 4 identical
    # selector blocks, one per chunk position within a 128-sample group)
    e32 = np.zeros((P, CHUNK), np.float16)
    for g in range(4):
        for s in range(CH_S):
            e32[g * CH_S + s, s::CH_S] = 1.0
    shared["e32"] = e32
    # per-core transposed obs, fp16, rows permuted agent-major per chunk
    obs_r = obs.reshape(NCORES, B_LOC // CH_S, CH_S, A, D)
    obs_r = obs_r.transpose(0, 1, 3, 2, 4)          # chunk, a, s
    obs_r = obs_r.reshape(NCORES, R_LOC, D).transpose(0, 2, 1)
    obs_r = np.ascontiguousarray(obs_r.astype(np.float16))
    in_maps = [dict(shared, obs=obs_r[k]) for k in range(NCORES)]
    return in_maps


def kernel(**inputs):
    if "nc" not in _CACHE:
        _CACHE["nc"] = _build()
    nc = _CACHE["nc"]
    in_maps = _host_prep(inputs)
    res = run_bass_kernel_spmd(nc, in_maps, core_ids=list(range(NCORES)))
    outs = np.stack([res.results[k]["out_t"] for k in range(NCORES)])
    # (8, 5, 16384) -> (8192, 16, 5), undoing the per-chunk agent-major
    # row permutation (r = a*CH_S + s within each 512-row chunk)
    out = outs.astype(np.float32).reshape(
        NCORES, NA, B_LOC // CH_S, A, CH_S)
    out = out.transpose(0, 2, 4, 3, 1)              # core, chunk, s, a, na
    return np.ascontiguousarray(out.reshape(B, A, NA)).astype(np.float32)


# revision 38
# speedup vs baseline: 1.0974x; 1.0390x over previous
"""TRN2 Bass kernel for nn_CNActorController (GRU comm-net actor).

Strategy (self-contained, shapes hardcoded):
- Pure data parallelism: batch dim sharded 8 ways (1024 samples / core).
- On-chip layout is feature-major ("transposed"): activations live as
  [feature partitions, row free-dim], rows ordered sample-major (b*16+a).
- The agent-mean communication step collapses (mask==const across agents):
    ci_n @ gru_k = (sum_a x)/16 @ gru_k + const
  so it is computed per-sample via a segmented row-sum + a small matmul,
  and all OU-noise terms + biases collapse into per-gate constants folded
  on the host.
- fp16 on-chip datatypes throughout (1 cyc/row on PE, 2x DVE mode),
  fp32 PSUM accumulation.
- Work is spread across all four compute engines: PE (matmuls), ACT
  (sigmoid/tanh/relu evacuations), DVE (gate combine arithmetic + some
  psum evacuations), Pool/GPSIMD (segment sums + broadcast adds).
"""
import os
import numpy as np

import jax
try:
    _cache_dir = os.path.expanduser("~/.cache/jax_kernel_cache")
    os.makedirs(_cache_dir, exist_ok=True)
    jax.config.update("jax_compilation_cache_dir", _cache_dir)
    jax.config.update("jax_persistent_cache_min_entry_size_bytes", -1)
    jax.config.update("jax_persistent_cache_min_compile_time_secs", 2)
except Exception:
    pass

import concourse.bass as bass
import concourse.mybir as mybir
import concourse.tile as tile
from concourse import bacc
from concourse.bass_utils import run_bass_kernel_spmd

# problem dims (hardcoded per spec)
B, A, D, H, C, NA = 8192, 16, 64, 256, 2, 5
NCORES = 8
B_LOC = B // NCORES          # 1024 samples per core
R_LOC = B_LOC * A            # 16384 rows per core
P = 128

SC_SAMPLES = 256             # samples per super-chunk
N_SC = B_LOC // SC_SAMPLES   # 4 super-chunks
CHUNK = 512                  # rows per chunk
CH_S = CHUNK // A            # 32 samples per chunk
N_CH = SC_SAMPLES * A // CHUNK  # 8 chunks per super-chunk
SC_ROWS = SC_SAMPLES * A     # 4096 rows per super-chunk

MT_G = 6                     # gate m-tiles (3H/128)
MT_H = 2                     # state m-tiles (H/128)
KT = 2                       # k-tiles for H contraction

f32 = mybir.dt.float32
f16 = mybir.dt.float16
AF = mybir.ActivationFunctionType
ALU = mybir.AluOpType

_CACHE = {}
N_REPS = 1

# engine placement knobs (tuned via timeline sim)
SEG_ENGINES = "pppp"         # segsum tree levels: p=pool, v=dve per level
HP_ENGINE = "dve"           # h-gate broadcast add: "pool" | "dve"
DEC_ENGINE = "split"         # dec relu evac: "dve" | "act" | "split"
CMT_ENGINE = "dve"           # cmt psum evac: "dve" | "act"
ENC2_ENGINE = "split"          # enc2 relu evac: "dve" | "act"
CM_L1_ENGINE = "act"         # layer-1 cm evac: "dve" | "act"
XOUT_L1_ENGINE = "dve"      # layer-1 final +h0: "pool" | "dve"
INTERLEAVE = True            # emit barrier blocks inside chunk loops
STREAM_OFF = 12              # slot offset between superchunk streams (24=serial)
SC_BUFS = 2                  # h0/x1 ring
TP_BUFS = 4                  # transient ring
PG_BUFS, PM_BUFS, PT_BUFS = 4, 2, 2   # psum bank split (sum*banks <= 8)


def _segsum_tree(nc, pool, engs, dst, src, tag):
    """dst[p, kt, s] = sum_a src[p, kt, s, a] via log2(A) adds.

    src is an AP [P, KT, A, S] (agent-major rows); engs is 4 engines, one
    per tree level (mix of nc.gpsimd / nc.vector — 2-byte packed operands
    let DVE run the adds in 2x mode, unlike tensor_reduce).
    """
    s = src.shape[3]
    t8 = pool.tile([P, KT, 8, s], f16, name=f"t8_{tag}", tag="segt8")
    t4 = pool.tile([P, KT, 4, s], f16, name=f"t4_{tag}", tag="segt4")
    t2 = pool.tile([P, KT, 2, s], f16, name=f"t2_{tag}", tag="segt2")
    engs[0].tensor_tensor(t8[:], src[:, :, 0:8, :], src[:, :, 8:16, :],
                          op=ALU.add)
    engs[1].tensor_tensor(t4[:], t8[:, :, 0:4, :], t8[:, :, 4:8, :],
                          op=ALU.add)
    engs[2].tensor_tensor(t2[:], t4[:, :, 0:2, :], t4[:, :, 2:4, :],
                          op=ALU.add)
    engs[3].tensor_tensor(dst, t2[:, :, 0, :], t2[:, :, 1, :], op=ALU.add)


def _build():
    """Build the per-core Bass module (same program for all 8 cores)."""
    nc = bacc.Bacc("TRN2", target_bir_lowering=False, debug=False,
                   num_devices=NCORES)

    # ---- DRAM I/O (all weights pre-converted to fp16 on host) ----
    obs_d = nc.dram_tensor("obs", [D, R_LOC], f16, kind="ExternalInput")
    wenc_d = nc.dram_tensor("wenc", [D, H], f16, kind="ExternalInput")
    benc_d = nc.dram_tensor("benc", [H], f32, kind="ExternalInput")
    wenc2_d = nc.dram_tensor("wenc2", [H, H], f16, kind="ExternalInput")
    benc2_d = nc.dram_tensor("benc2", [H], f32, kind="ExternalInput")
    grk_d = nc.dram_tensor("grk", [C, H, 3 * H], f16, kind="ExternalInput")
    gks_d = nc.dram_tensor("gks", [C, H, 3 * H], f16, kind="ExternalInput")
    cmb_d = nc.dram_tensor("cmb", [C, 3 * H], f32, kind="ExternalInput")
    b1h_d = nc.dram_tensor("b1h", [C, H], f32, kind="ExternalInput")
    wdec_d = nc.dram_tensor("wdec", [H, H], f16, kind="ExternalInput")
    bdec_d = nc.dram_tensor("bdec", [H], f32, kind="ExternalInput")
    wout_d = nc.dram_tensor("wout", [H, NA], f16, kind="ExternalInput")
    bout_d = nc.dram_tensor("bout", [NA, 1], f32, kind="ExternalInput")
    e32_d = nc.dram_tensor("e32", [P, CHUNK], f16, kind="ExternalInput")
    out_d = nc.dram_tensor("out_t", [NA, R_LOC], f16, kind="ExternalOutput")

    with tile.TileContext(nc) as tc:
        with (
            tc.tile_pool(name="wp", bufs=1) as wp,          # weights, consts
            tc.tile_pool(name="sc", bufs=SC_BUFS) as scp,   # per-superchunk h0
            tc.tile_pool(name="xp", bufs=SC_BUFS) as xp,    # x state buffer
            tc.tile_pool(name="sp", bufs=2) as sp,          # segsums + cm
            tc.tile_pool(name="tp", bufs=TP_BUFS) as tp,    # chunk transients
            tc.tile_pool(name="pg", bufs=PG_BUFS, space="PSUM") as pg,
            tc.tile_pool(name="pm", bufs=PM_BUFS, space="PSUM") as pm,
            tc.tile_pool(name="pt", bufs=PT_BUFS, space="PSUM") as pt,
        ):
            # "mm" ring: enc/enc2/dec/out evac psums; "bar" ring: layer
            # boundary (cm_h blocks + cmt transposes); gates get their own
            # ring so the PE can run ahead of slow evacuations.
            def psum_mm(name):
                return pm.tile([P, CHUNK], f32, name=name, tag="mm")

            def psum_bar(name):
                return pt.tile([P, CHUNK], f32, name=name, tag="bar")
            # ---- load weights (direct fp16 DMA, no rounding copies) ----
            wenc = wp.tile([D, H], f16, name="wenc")
            nc.sync.dma_start(wenc[:], wenc_d[:])
            wenc2 = wp.tile([P, KT, H], f16, name="wenc2")
            nc.sync.dma_start(wenc2[:],
                              wenc2_d.rearrange("(kt p) m -> p kt m", p=P))
            grk = wp.tile([P, C, KT, 3 * H], f16, name="grk")
            gks = wp.tile([P, C, KT, 3 * H], f16, name="gks")
            for li in range(C):
                nc.sync.dma_start(
                    grk[:, li], grk_d[li].rearrange("(kt p) m -> p kt m", p=P))
                nc.sync.dma_start(
                    gks[:, li], gks_d[li].rearrange("(kt p) m -> p kt m", p=P))
            wdec = wp.tile([P, KT, H], f16, name="wdec")
            nc.sync.dma_start(wdec[:],
                              wdec_d.rearrange("(kt p) m -> p kt m", p=P))
            wout = wp.tile([P, KT, NA], f16, name="wout")
            nc.sync.dma_start(wout[:],
                              wout_d.rearrange("(kt p) m -> p kt m", p=P))

            benc = wp.tile([P, MT_H], f32, name="benc")
            nc.sync.dma_start(benc[:], benc_d.rearrange("(mt p) -> p mt", p=P))
            benc2 = wp.tile([P, MT_H], f32, name="benc2")
            nc.sync.dma_start(benc2[:], benc2_d.rearrange("(mt p) -> p mt", p=P))
            bdec = wp.tile([P, MT_H], f32, name="bdec")
            nc.sync.dma_start(bdec[:], bdec_d.rearrange("(mt p) -> p mt", p=P))
            bout = wp.tile([NA, 1], f32, name="bout")
            nc.sync.dma_start(bout[:], bout_d[:])
            cmb = wp.tile([P, C, MT_G], f32, name="cmb")
            nc.sync.dma_start(cmb[:], cmb_d.rearrange("c (mt p) -> p c mt", p=P))
            b1h = wp.tile([P, C, MT_H], f32, name="b1h")
            nc.sync.dma_start(b1h[:], b1h_d.rearrange("c (mt p) -> p c mt", p=P))
            e32 = wp.tile([P, CHUNK], f16, name="e32")
            nc.sync.dma_start(e32[:], e32_d[:])

            # ---- main loop over super-chunks (software-pipelined
            # emission: engines execute their streams in order, so each
            # layer's cm/cmt "barrier" blocks are emitted right after the
            # two chunks they depend on) ----
            BS = 2 * CH_S            # 64 samples per barrier block
            seg_engs = [nc.gpsimd if c == "p" else nc.vector
                        for c in SEG_ENGINES]

            def emit_phase1_chunk(sc, ctx, ch):
                h0, s1 = ctx["h0"], ctx["s1"]
                r0 = sc * SC_ROWS + ch * CHUNK   # global row offset
                co = ch * CHUNK                  # row offset within sc
                obs_t = tp.tile([D, CHUNK], f16, name="obs_t")
                nc.sync.dma_start(obs_t[:], obs_d[:, r0:r0 + CHUNK])
                # enc (K=64), then enc2 (K=256)
                x1t = tp.tile([P, MT_H, CHUNK], f16, name="x1t", tag="mlp_tmp")
                for mt in range(MT_H):
                    pse = psum_mm("pse")
                    nc.tensor.matmul(
                        pse[:], wenc[:, mt * P:(mt + 1) * P], obs_t[:],
                        start=True, stop=True)
                    nc.scalar.activation(
                        x1t[:, mt, :], pse[:], AF.Relu, bias=benc[:, mt:mt + 1])
                for mt in range(MT_H):
                    pse2 = psum_mm("pse2")
                    for kt in range(KT):
                        nc.tensor.matmul(
                            pse2[:], wenc2[:, kt, mt * P:(mt + 1) * P],
                            x1t[:, kt, :], start=(kt == 0), stop=(kt == KT - 1))
                    e2_dve = (ENC2_ENGINE == "dve" or
                              (ENC2_ENGINE == "split" and mt == 1))
                    if e2_dve:
                        nc.vector.tensor_scalar(
                            h0[:, mt, co:co + CHUNK], pse2[:],
                            benc2[:, mt:mt + 1], 0.0, op0=ALU.add, op1=ALU.max)
                    else:
                        nc.scalar.activation(
                            h0[:, mt, co:co + CHUNK], pse2[:], AF.Relu,
                            bias=benc2[:, mt:mt + 1])
                # segsum of h0 chunk -> s1 (tree levels split Pool/DVE)
                _segsum_tree(
                    nc, tp, seg_engs,
                    s1[:, :, ch * CH_S:(ch + 1) * CH_S],
                    h0[:, :, co:co + CHUNK].rearrange(
                        "p mt (a s) -> p mt a s", s=CH_S),
                    tag="s1")

            def emit_barrier_block(ctx, li, blk):
                """cm_h block for 64 samples; every second call also emits
                the transposed z,r cm for a packed 128-sample group."""
                sseg = ctx["s1"] if li == 0 else ctx["s2"]
                cm = ctx["cm"][li]
                psc = psum_bar("psc")
                for mt in range(MT_H):
                    for kt in range(KT):
                        nc.tensor.matmul(
                            psc[:, mt * BS:(mt + 1) * BS],
                            gks[:, li, kt, (4 + mt) * P:(5 + mt) * P],
                            sseg[:, kt, blk * BS:(blk + 1) * BS],
                            start=(kt == 0), stop=(kt == KT - 1))
                    if li == 0 or CM_L1_ENGINE == "act":
                        nc.scalar.activation(
                            cm[:, mt, blk * BS:(blk + 1) * BS],
                            psc[:, mt * BS:(mt + 1) * BS], AF.Identity,
                            bias=cmb[:, li, 4 + mt:5 + mt])
                    else:
                        nc.vector.tensor_scalar(
                            cm[:, mt, blk * BS:(blk + 1) * BS],
                            psc[:, mt * BS:(mt + 1) * BS],
                            cmb[:, li, 4 + mt:5 + mt], None, op0=ALU.add)
                if blk % 2 == 1:
                    # transposed cm for z,r: 128 samples per stationary load
                    # (evacuated as two 64-partition tiles; PE APs only
                    # support base partitions 0/32/64)
                    g4 = blk // 2
                    pct = psum_bar("pct")
                    for kt in range(KT):
                        nc.tensor.matmul(
                            pct[:],
                            sseg[:, kt, g4 * P:(g4 + 1) * P],
                            gks[:, li, kt, 0:2 * H],
                            start=(kt == 0), stop=(kt == KT - 1))
                    for half in range(2):
                        cmt = sp.tile([64, 2 * H], f16, name="cmt", tag="cmt",
                                      bufs=8)
                        if CMT_ENGINE == "dve":
                            nc.vector.tensor_copy(
                                cmt[:], pct[half * 64:(half + 1) * 64, :])
                        else:
                            nc.scalar.copy(
                                cmt[:], pct[half * 64:(half + 1) * 64, :])
                        ctx["cmts"][li].append(cmt)

            def emit_gru_chunk(sc, ctx, li, ch):
                h0, s1, s2, x1 = ctx["h0"], ctx["s1"], ctx["s2"], ctx["x1"]
                cm, cmts = ctx["cm"][li], ctx["cmts"][li]
                xin = h0 if li == 0 else x1
                xout = x1
                co = ch * CHUNK
                ss = ch * CH_S
                zt = tp.tile([P, MT_H, CHUNK], f16, name="zt")
                rt = tp.tile([P, MT_H, CHUNK], f16, name="rt")
                hh = tp.tile([P, MT_H, CHUNK], f16, name="hh")
                # gate psums: mh = x @ gru_rk.  m-tile order r,h,z: the
                # critical chain runs r-sigmoid -> rrh -> hp -> tanh ->
                # combine, so r first and z (only needed at the combine
                # multiply) last.
                rrh = tp.tile([P, MT_H, CHUNK], f16, name="rrh")
                for mt in (2, 3, 4, 5, 0, 1):
                    psg = pg.tile([P, CHUNK], f32, name="psg")
                    for kt in range(KT):
                        nc.tensor.matmul(
                            psg[:], grk[:, li, kt, mt * P:(mt + 1) * P],
                            xin[:, kt, co:co + CHUNK],
                            start=(kt == 0), stop=(kt == KT - 1 and mt >= 4))
                    if mt < 4:
                        # z,r: cm broadcast via expander matmul into psum,
                        # then sigmoid w/ folded const bias
                        po = 32 * (ch % 2)
                        nc.tensor.matmul(
                            psg[:],
                            cmts[ch // 2][po:po + 32, mt * P:(mt + 1) * P],
                            e32[po:po + 32, :], start=False, stop=True)
                        dst = zt if mt < 2 else rt
                        nc.scalar.activation(
                            dst[:, mt % 2, :], psg[:], AF.Sigmoid,
                            bias=cmb[:, li, mt:mt + 1])
                    else:
                        # h gate: rrh = r * (mh_h + b1h) per m-tile
                        mtl = mt - 4
                        nc.vector.scalar_tensor_tensor(
                            rrh[:, mtl, :], psg[:], b1h[:, li, mtl:mtl + 1],
                            rt[:, mtl, :], op0=ALU.add, op1=ALU.mult)
                # merged across both h m-tiles: one broadcast add on Pool
                # + one tanh on ACT
                cmv2 = cm[:, :, ss:ss + CH_S].unsqueeze(
                    2).broadcast_to([P, MT_H, A, CH_S])
                hp = tp.tile([P, MT_H, CHUNK], f16, name="hp")
                eng_hp = nc.gpsimd if HP_ENGINE == "pool" else nc.vector
                eng_hp.tensor_tensor(
                    hp.rearrange("p mt (a s) -> p mt a s", s=CH_S),
                    rrh.rearrange("p mt (a s) -> p mt a s", s=CH_S),
                    cmv2, op=ALU.add)
                nc.scalar.activation(hh[:], hp[:], AF.Tanh)
                # combine: x_new = hh + z*(x - hh) + h0
                dtl = tp.tile([P, MT_H, CHUNK], f16, name="dtl")
                nc.vector.tensor_tensor(
                    dtl[:], xin[:, :, co:co + CHUNK], hh[:], op=ALU.subtract)
                nc.vector.tensor_tensor(dtl[:], zt[:], dtl[:], op=ALU.mult)
                nc.vector.tensor_tensor(dtl[:], dtl[:], hh[:], op=ALU.add)
                nc.vector.tensor_tensor(
                    xout[:, :, co:co + CHUNK], dtl[:],
                    h0[:, :, co:co + CHUNK], op=ALU.add)
                if li == 0:
                    _segsum_tree(
                        nc, tp, seg_engs,
                        s2[:, :, ss:ss + CH_S],
                        xout[:, :, co:co + CHUNK].rearrange(
                            "p mt (a s) -> p mt a s", s=CH_S),
                        tag="s2")
                else:
                    # ---- dec + out fused ----
                    dd = tp.tile([P, MT_H, CHUNK], f16, name="dd")
                    for mt in range(MT_H):
                        psd = psum_mm("psd")
                        for kt in range(KT):
                            nc.tensor.matmul(
                                psd[:], wdec[:, kt, mt * P:(mt + 1) * P],
                                xout[:, kt, co:co + CHUNK],
                                start=(kt == 0), stop=(kt == KT - 1))
                        dec_act = (DEC_ENGINE == "act" or
                                   (DEC_ENGINE == "split" and mt == 0))
                        if dec_act:
                            nc.scalar.activation(
                                dd[:, mt, :], psd[:], AF.Relu,
                                bias=bdec[:, mt:mt + 1])
                        else:
                            nc.vector.tensor_scalar(
                                dd[:, mt, :], psd[:], bdec[:, mt:mt + 1], 0.0,
                                op0=ALU.add, op1=ALU.max)
                    pso = pm.tile([NA, CHUNK], f32, name="pso", tag="mm")
                    for kt in range(KT):
                        nc.tensor.matmul(
                            pso[:], wout[:, kt, :], dd[:, kt, :],
                            start=(kt == 0), stop=(kt == KT - 1))
                    ot = tp.tile([NA, CHUNK], f16, name="ot")
                    nc.scalar.add(ot[:], pso[:], bout[:])
                    nc.sync.dma_start(
                        out_d[:, sc * SC_ROWS + co:sc * SC_ROWS + co + CHUNK],
                        ot[:])

            import contextlib
            rep_ctx = (tc.For_i(0, N_REPS, 1) if N_REPS > 1
                       else contextlib.nullcontext())
            def make_ctx():
                return {
                    "h0": scp.tile([P, MT_H, SC_ROWS], f16, name="h0",
                                   tag="h0"),
                    "s1": sp.tile([P, KT, SC_SAMPLES], f16, name="s1",
                                  tag="seg1"),
                    "s2": sp.tile([P, KT, SC_SAMPLES], f16, name="s2",
                                  tag="seg2"),
                    "x1": xp.tile([P, MT_H, SC_ROWS], f16, name="x1",
                                  tag="x"),
                    "cm": [sp.tile([P, MT_H, SC_SAMPLES], f16,
                                   name=f"cm{li}", tag="cm", bufs=4)
                           for li in range(C)],
                    "cmts": [[], []],
                }

            def emit_step(sc, ctx, s):
                # 24 steps per superchunk: 8 phase-1, 8 layer-0, 8 layer-1;
                # barrier blocks ride the odd steps of the preceding phase.
                ph, ch = divmod(s, N_CH)
                if ph == 0:
                    emit_phase1_chunk(sc, ctx, ch)
                    if ch % 2 == 1:
                        emit_barrier_block(ctx, 0, (ch - 1) // 2)
                elif ph == 1:
                    emit_gru_chunk(sc, ctx, 0, ch)
                    if ch % 2 == 1:
                        emit_barrier_block(ctx, 1, (ch - 1) // 2)
                else:
                    emit_gru_chunk(sc, ctx, 1, ch)

            N_STEP = 3 * N_CH
            with rep_ctx:
              ctxs = {}
              for t in range((N_SC - 1) * STREAM_OFF + N_STEP):
                  for sc in range(N_SC):
                      s = t - sc * STREAM_OFF
                      if s == 0:
                          ctxs[sc] = make_ctx()
                      if 0 <= s < N_STEP:
                          emit_step(sc, ctxs[sc], s)

    nc.compile()
    return nc


def _host_prep(inputs):
    """Host-side preprocessing of weights/constants (tiny, O(H^2))."""
    g = lambda k: np.asarray(inputs[k], np.float32)
    obs = g("obs")
    mask = g("mask")            # (1, A, 1)
    ou_s0, ou_s1 = g("ou_s0"), g("ou_s1")   # (C,1,A,H)
    ou_s2, ou_s3 = g("ou_s2"), g("ou_s3")   # (C,1,1,H)
    gru_k, gru_b = g("gru_k"), g("gru_b")

    m = mask[0, :, :]                        # (A, 1)
    cmb = np.zeros((C, 3 * H), np.float32)
    b1h = np.zeros((C, H), np.float32)
    gks = np.zeros((C, H, 3 * H), np.float32)
    for i in range(C):
        send = (m * ou_s1[i, 0] * ou_s0[i, 0]).sum(0) / A      # (H,)
        recv = (m.mean(0) * ou_s3[i, 0, 0] * ou_s2[i, 0, 0])   # (H,)
        const = (send + recv).astype(np.float64) @ gru_k[i].astype(np.float64)
        cmbi = const + gru_b[i, 0].astype(np.float64)
        cmbi[:2 * H] += gru_b[i, 1, :2 * H].astype(np.float64)
        cmb[i] = cmbi.astype(np.float32)
        b1h[i] = gru_b[i, 1, 2 * H:]
        gks[i] = gru_k[i] / A

    h16 = lambda x: np.ascontiguousarray(x.astype(np.float16))
    shared = {
        "wenc": h16(g("W_enc")), "benc": g("b_enc"),
        "wenc2": h16(g("W_enc2")), "benc2": g("b_enc2"),
        "grk": h16(g("gru_rk")), "gks": h16(gks), "cmb": cmb, "b1h": b1h,
        "wdec": h16(g("W_dec")), "bdec": g("b_dec"),
        "wout": h16(g("W_out")), "bout": g("b_out").reshape(NA, 1),
    }
    # expander matrix: broadcast per-sample rows to per-row positions
    # (rows within a chunk are agent-major: r = a*CH_S + s; 4 identical
    # selector blocks, one per chunk position within a 128-sample group)
    e32 = np.zeros((P, CHUNK), np.float16)
    for g in range(4):
        for s in range(CH_S):
            e32[g * CH_S + s, s::CH_S] = 1.0
    shared["e32"] = e32
    # per-core transposed obs, fp16, rows permuted agent-major per chunk
    obs_r = obs.reshape(NCORES, B_LOC // CH_S, CH_S, A, D)
    obs_r = obs_r.transpose(0, 1, 3, 2, 4)          # chunk, a, s
    obs_r = obs_r.reshape(NCORES, R_LOC, D).transpose(0, 2, 1)
    obs_r = np.ascontiguousarray(obs_r.astype(np.float16))
    in_maps = [dict(shared, obs=obs_r[k]) for k in range(NCORES)]
    return in_maps


def kernel(**inputs):
    if "nc" not in _CACHE:
        _CACHE["nc"] = _build()
    nc = _CACHE["nc"]
    in_maps = _host_prep(inputs)
    res = run_bass_kernel_spmd(nc, in_maps, core_ids=list(range(NCORES)))
    outs = np.stack([res.results[k]["out_t"] for k in range(NCORES)])
    # (8, 5, 16384) -> (8192, 16, 5), undoing the per-chunk agent-major
    # row permutation (r = a*CH_S + s within each 512-row chunk)
    out = outs.astype(np.float32).reshape(
        NCORES, NA, B_LOC // CH_S, A, CH_S)
    out = out.transpose(0, 2, 4, 3, 1)              # core, chunk, s, a, na
    return np.ascontiguousarray(out.reshape(B, A, NA)).astype(np.float32)
